# revision 16
# baseline (speedup 1.0000x reference)
"""LorentzMoE (top-1 routing, E=8 experts) on 8 Trainium2 NeuronCores.

Strategy (expert-parallel, host control plane):
  - Host computes the gate (softmax + bias + top-1) in numpy, exactly
    mirroring the reference numerics.
  - Tokens are dispatched by expert: core c gets every token routed to
    expert c (padded to a uniform capacity so one SPMD NEFF serves all
    8 cores).  Core c also computes the shared expert for those same
    tokens, the LResNet combine and the Lorentz normalization, so each
    token's full output is produced on a single core.
  - The gate weight is folded into the expert FFN: the W3 matmul
    consumes x pre-scaled by (2*w_tok) on the host, making the routed
    space output 2*w*o directly; the routed time component is
    sqrt(colsum(o'^2) + (2w)^2) which only needs a per-token row.
  - Host scatters per-core outputs back to the original token order.

Device layout: feature-on-partition ("transposed") everywhere.
  Weights are packed host-side as [128, 8, 1024] (partition-major) and
  DMAed as four quarter-matrices spread over four HWDGE queues
  (sync/gpsimd/scalar/vector) so packets are 4KB-contiguous per
  partition and aggregate DMA bandwidth is available from t=0.
  Column sums (over partitions) use ones-vector matmuls accumulated
  across the 8 chunks in PSUM; they are deferred out of the up-proj
  phases so PSUM stays shallow, and the |comb|^2 sum is folded into
  the shared-expert down-proj loop to avoid a serial epilogue.
"""

import numpy as np

D = 1024
E = 8
NCH = 8  # 1024 / 128 partition chunks
SCALE = 2.0

_cache: dict = {}


def _round_up(v, m):
    return ((v + m - 1) // m) * m


def _host_gate(x, gate_w, gate_b):
    """Replicates the reference gate in f32 numpy (verified bit-identical
    top-1 selection vs the jax reference on the benchmark inputs)."""
    logits = (x[:, 1:] @ gate_w.T).astype(np.float32)
    m = logits.max(-1, keepdims=True)
    e = np.exp(logits - m, dtype=np.float32)
    scores = e / e.sum(-1, keepdims=True, dtype=np.float32)
    biased = scores + gate_b
    idx = np.argmax(biased, axis=-1)
    w = scores[np.arange(x.shape[0]), idx]
    return idx.astype(np.int64), w.astype(np.float32)


def _build_nc(cap):
    import concourse.mybir as mybir
    import concourse.tile as tile
    from concourse import bacc

    f32 = mybir.dt.float32
    bf16 = mybir.dt.bfloat16
    AF = mybir.ActivationFunctionType
    ALU = mybir.AluOpType

    nc = bacc.Bacc("TRN2", target_bir_lowering=False, debug=False)

    # ---- DRAM I/O (weights packed [128, NCH, 1024]; x [128, NCH, cap]) ----
    xw_d = nc.dram_tensor("xw", [128, NCH, cap], bf16, kind="ExternalInput")
    xs_d = nc.dram_tensor("xs", [128, NCH, cap], bf16, kind="ExternalInput")
    w1_d = nc.dram_tensor("w1t", [128, NCH, D], bf16, kind="ExternalInput")
    w3_d = nc.dram_tensor("w3t", [128, NCH, D], bf16, kind="ExternalInput")
    w2_d = nc.dram_tensor("w2t", [128, NCH, D], bf16, kind="ExternalInput")
    v1_d = nc.dram_tensor("v1t", [128, NCH, D], bf16, kind="ExternalInput")
    v3_d = nc.dram_tensor("v3t", [128, NCH, D], bf16, kind="ExternalInput")
    v2_d = nc.dram_tensor("v2t", [128, NCH, D], bf16, kind="ExternalInput")
    r4_d = nc.dram_tensor("r4w2", [128, cap], f32, kind="ExternalInput")
    out_d = nc.dram_tensor("outT", [128, NCH, cap], f32, kind="ExternalOutput")

    with tile.TileContext(nc) as tc:
        with (
            tc.tile_pool(name="consts", bufs=1) as consts,
            tc.tile_pool(name="xpool", bufs=1) as xpool,
            tc.tile_pool(name="wpool", bufs=4) as wpool,
            tc.tile_pool(name="hpool", bufs=1) as hpool,
            tc.tile_pool(name="work", bufs=3) as work,
            tc.tile_pool(name="rows", bufs=1) as rows,
            tc.tile_pool(name="psum", bufs=8, space="PSUM") as psum,
        ):
            QS = [nc.sync, nc.gpsimd, nc.scalar]

            ones_mat = consts.tile([128, 128], f32)
            nc.vector.memset(ones_mat, 1.0)

            # x first on each queue so the first matmuls have operands
            xw_a = xpool.tile([128, 4, cap], bf16)
            QS[0].dma_start(out=xw_a, in_=xw_d[:, 0:4, :])
            xw_b = xpool.tile([128, 4, cap], bf16)
            QS[1].dma_start(out=xw_b, in_=xw_d[:, 4:8, :])
            xs_a = xpool.tile([128, 4, cap], bf16)
            QS[2].dma_start(out=xs_a, in_=xs_d[:, 0:4, :])
            xs_b = xpool.tile([128, 4, cap], bf16)
            QS[2].dma_start(out=xs_b, in_=xs_d[:, 4:8, :])
            xw_sb = [(xw_a if k < 4 else xw_b)[:, k % 4, :] for k in range(NCH)]
            xs_sb = [(xs_a if k < 4 else xs_b)[:, k % 4, :] for k in range(NCH)]

            r4_sb = rows.tile([128, cap], f32)
            QS[2].dma_start(out=r4_sb, in_=r4_d[:, :])

            _w_count = [0]

            def load_w(dram, nm):
                """Four quarter-matrix tiles spread over the HWDGE queues."""
                qt = []
                off = _w_count[0]
                _w_count[0] += 1
                for q in range(4):
                    t = wpool.tile(
                        [128, 2, D], bf16, name=f"{nm}q{q}", tag=f"wq{q}"
                    )
                    QS[(q + off) % 3].dma_start(
                        out=t, in_=dram[:, 2 * q : 2 * q + 2, :]
                    )
                    qt.append(t)

                def sl(k, m):
                    return qt[k // 2][:, k % 2, 128 * m : 128 * (m + 1)]

                return sl

            # persistent activations
            h_r = [
                hpool.tile([128, cap], bf16, name=f"hr{k}", tag=f"hr{k}")
                for k in range(NCH)
            ]
            h_s = [
                hpool.tile([128, cap], bf16, name=f"hs{k}", tag=f"hs{k}")
                for k in range(NCH)
            ]
            oc = hpool.tile([128, NCH, cap], f32)

            def up_proj(wa, wb, xa, xb, h_out, interleave=False):
                """h_out[m] = bf16( silu(wa.T@xa) * (wb.T@xb) ) per chunk m.
                interleave=True orders the k-loops by DMA quarter arrival
                so the first phase can start before all weights landed."""
                for m in range(NCH):
                    ps1 = psum.tile([128, cap], f32, name=f"ps1_{m}", tag="mm")
                    ps3 = psum.tile([128, cap], f32, name=f"ps3_{m}", tag="mm")
                    if interleave:
                        for k in range(4):
                            nc.tensor.matmul(
                                ps1, wa(k, m), xa[k], start=(k == 0), stop=False,
                                skip_group_check=True,
                            )
                        for k in range(4):
                            nc.tensor.matmul(
                                ps3, wb(k, m), xb[k], start=(k == 0), stop=False,
                                skip_group_check=True,
                            )
                        for k in range(4, NCH):
                            nc.tensor.matmul(
                                ps1, wa(k, m), xa[k], start=False,
                                stop=(k == NCH - 1), skip_group_check=True,
                            )
                        for k in range(4, NCH):
                            nc.tensor.matmul(
                                ps3, wb(k, m), xb[k], start=False,
                                stop=(k == NCH - 1), skip_group_check=True,
                            )
                    else:
                        for k in range(NCH):
                            nc.tensor.matmul(
                                ps1, wa(k, m), xa[k], start=(k == 0),
                                stop=(k == NCH - 1),
                            )
                        for k in range(NCH):
                            nc.tensor.matmul(
                                ps3, wb(k, m), xb[k], start=(k == 0),
                                stop=(k == NCH - 1),
                            )
                    m1 = work.tile([128, cap], f32, name=f"m1_{m}", tag="m1")
                    nc.scalar.activation(m1, ps1, AF.Silu)
                    nc.vector.tensor_mul(h_out[m], m1, ps3)

            def colsum_sq(h_in, ps_t):
                """ps_t [128,cap] = colsum(h_in^2) replicated on all parts."""
                for m in range(NCH):
                    sq = work.tile([128, cap], f32, name=f"sq_{m}", tag="sq")
                    nc.vector.tensor_mul(sq, h_in[m], h_in[m])
                    nc.tensor.matmul(
                        ps_t,
                        ones_mat,
                        sq,
                        start=(m == 0),
                        stop=(m == NCH - 1),
                        skip_group_check=True,
                    )

            def down_proj(w2, h_in, ps_o, ps_li=None):
                """oc[:,m,:] (+)= w2.T @ h_in ; ps_o = colsum(o^2) replicated;
                if ps_li given (shared pass): oc += o and ps_li = colsum(oc^2)."""
                for m in range(NCH):
                    po = psum.tile([128, cap], f32, name=f"po_{m}", tag="mm")
                    ks = list(range(1, NCH)) + [0]
                    for j, k in enumerate(ks):
                        nc.tensor.matmul(
                            po, w2(k, m), h_in[k], start=(j == 0), stop=(j == NCH - 1)
                        )
                    if ps_li is not None:
                        sq = work.tile([128, cap], f32, name=f"sqo_{m}", tag="sq")
                        nc.scalar.square(sq, po)
                        nc.vector.tensor_add(oc[:, m, :], oc[:, m, :], po)
                        sqc = work.tile([128, cap], f32, name=f"sqc_{m}", tag="sqc")
                        nc.vector.tensor_mul(sqc, oc[:, m, :], oc[:, m, :])
                    else:
                        nc.scalar.copy(oc[:, m, :], po)
                        sq = work.tile([128, cap], f32, name=f"sqo_{m}", tag="sq")
                        nc.vector.tensor_mul(sq, oc[:, m, :], oc[:, m, :])
                    nc.tensor.matmul(
                        ps_o,
                        ones_mat,
                        sq,
                        start=(m == 0),
                        stop=(m == NCH - 1),
                        skip_group_check=True,
                    )
                    if ps_li is not None:
                        nc.tensor.matmul(
                            ps_li,
                            ones_mat,
                            sqc,
                            start=(m == 0),
                            stop=(m == NCH - 1),
                            skip_group_check=True,
                        )

            # ================= routed expert =================
            w1 = load_w(w1_d, "w1")
            w3 = load_w(w3_d, "w3")
            up_proj(w1, w3, xw_sb, xs_sb, h_r, interleave=True)
            w2 = load_w(w2_d, "w2")
            # t' = sqrt(colsum(sp'^2) + (2w)^2) -> bf16 row 0 of h_r[0]
            ps_tr = psum.tile([128, cap], f32, name="ps_tr", tag="mm")
            colsum_sq(h_r, ps_tr)
            trow = rows.tile([1, cap], f32)
            nc.vector.tensor_add(trow, ps_tr[0:1, :], r4_sb[0:1, :])
            nc.scalar.activation(h_r[0][0:1, :], trow, AF.Sqrt)

            ps_or = psum.tile([128, cap], f32, name="ps_or", tag="mm")
            down_proj(w2, h_r, ps_or)

            # ================= shared expert =================
            v1 = load_w(v1_d, "v1")
            v3 = load_w(v3_d, "v3")
            up_proj(v1, v3, xw_sb, xw_sb, h_s)
            v2 = load_w(v2_d, "v2")
            ps_ts = psum.tile([128, cap], f32, name="ps_ts", tag="mm")
            colsum_sq(h_s, ps_ts)
            nc.scalar.activation(h_s[0][0:1, :], ps_ts[0:1, :], AF.Sqrt, bias=1.0)

            ps_os = psum.tile([128, cap], f32, name="ps_os", tag="mm")
            ps_li = psum.tile([128, cap], f32, name="ps_li", tag="mm")
            down_proj(v2, h_s, ps_os, ps_li=ps_li)

            # ================= combine + Lorentz normalize =================
            # (all row quantities replicated across 128 partitions)
            # comb_space = o_shared + 2w*o_routed  (already summed in oc)
            # comb_time  = ot_shared + 2 + 2w*ot_routed
            a_or = rows.tile([128, cap], f32)
            nc.vector.tensor_add(a_or, ps_or, r4_sb)
            orow = rows.tile([128, cap], f32)
            nc.scalar.activation(orow, a_or, AF.Sqrt)  # = 2w*ot_routed
            osrow = rows.tile([128, cap], f32)
            nc.scalar.activation(osrow, ps_os, AF.Sqrt, bias=1.0)  # ot_shared
            ct = rows.tile([128, cap], f32)
            nc.vector.scalar_tensor_tensor(
                ct, orow, 2.0, osrow, op0=ALU.add, op1=ALU.add
            )
            ct2 = rows.tile([128, cap], f32)
            nc.vector.tensor_mul(ct2, ct, ct)
            absli = rows.tile([128, cap], f32)
            nc.vector.tensor_sub(absli, ct2, ps_li)  # = |<comb,comb>_L| > 0
            sqli = rows.tile([128, cap], f32)
            nc.scalar.activation(sqli, absli, AF.Sqrt)
            inv = rows.tile([128, cap], f32)
            nc.vector.reciprocal(inv, sqli)
            otime = rows.tile([1, cap], f32)
            nc.vector.tensor_mul(otime, ct[0:1, :], inv[0:1, :])

            # batched scale by inv (free-dim broadcast), split for DMA overlap
            import concourse.bass as bass

            inv_b4 = bass.AP(
                tensor=inv.tensor,
                offset=inv.offset,
                ap=[inv.ap[0], [0, 4], inv.ap[1]],
            )
            nc.vector.tensor_mul(oc[:, 0:4, :], oc[:, 0:4, :], inv_b4)
            nc.vector.tensor_copy(oc[0:1, 0, :], otime)
            nc.sync.dma_start(out=out_d[:, 0:4, :], in_=oc[:, 0:4, :])
            nc.vector.tensor_mul(oc[:, 4:8, :], oc[:, 4:8, :], inv_b4)
            nc.gpsimd.dma_start(out=out_d[:, 4:8, :], in_=oc[:, 4:8, :])

    nc.compile()
    return nc


def _get_nc(cap):
    if cap not in _cache:
        _cache[cap] = _build_nc(cap)
    return _cache[cap]


def _pack_w(mat_t):
    """[1024, 1024] (K, M) -> [128, 8, 1024] partition-major bf16."""
    return np.ascontiguousarray(mat_t.reshape(NCH, 128, D).transpose(1, 0, 2))


def kernel(x, gate_w, gate_b, W1, W3, W2, Ws1, Ws3, Ws2):
    import ml_dtypes

    from concourse.bass_utils import run_bass_kernel_spmd

    bf16 = ml_dtypes.bfloat16
    x = np.asarray(x, dtype=np.float32)
    gate_w = np.asarray(gate_w, dtype=np.float32)
    gate_b = np.asarray(gate_b, dtype=np.float32)
    W1 = np.asarray(W1, dtype=np.float32)
    W3 = np.asarray(W3, dtype=np.float32)
    W2 = np.asarray(W2, dtype=np.float32)
    Ws1 = np.asarray(Ws1, dtype=np.float32)
    Ws3 = np.asarray(Ws3, dtype=np.float32)
    Ws2 = np.asarray(Ws2, dtype=np.float32)

    T = x.shape[0]
    idx, w = _host_gate(x, gate_w, gate_b)
    counts = np.bincount(idx, minlength=E)
    cap = max(64, _round_up(int(counts.max()), 8))
    assert cap <= 512, f"capacity {cap} exceeds single-block limit"

    toks = [np.where(idx == c)[0] for c in range(E)]

    # host-side weight prep: transposed [K, M] with zero col for the time
    # slot, then packed partition-major
    def prep(Wm):  # [A, B] -> [B, A+1] bf16 (col 0 zero), packed
        out = np.zeros((D, D), dtype=bf16)
        out[:, 1:] = Wm.T.astype(bf16)
        return _pack_w(out)

    v1t = prep(Ws1)
    v3t = prep(Ws3)
    v2t = prep(Ws2)

    in_maps = []
    for c in range(E):
        tc_ = toks[c]
        n = len(tc_)
        xt = np.zeros((NCH, 128, cap), dtype=bf16)
        xt.reshape(D, cap)[:, :n] = x[tc_].T.astype(bf16)
        xs = np.zeros((NCH, 128, cap), dtype=bf16)
        tw = (SCALE * w[tc_]).astype(np.float32)
        xs.reshape(D, cap)[:, :n] = (x[tc_] * tw[:, None]).T.astype(bf16)
        r4row = np.zeros((cap,), dtype=np.float32)
        r4row[:n] = tw * tw
        r4 = np.ascontiguousarray(np.broadcast_to(r4row, (128, cap)))
        in_maps.append(
            {
                "xw": np.ascontiguousarray(xt.transpose(1, 0, 2)),
                "xs": np.ascontiguousarray(xs.transpose(1, 0, 2)),
                "w1t": prep(W1[c]),
                "w3t": prep(W3[c]),
                "w2t": prep(W2[c]),
                "v1t": v1t,
                "v3t": v3t,
                "v2t": v2t,
                "r4w2": r4,
            }
        )

    nc = _get_nc(cap)
    res = run_bass_kernel_spmd(nc, in_maps, core_ids=list(range(E)))

    out = np.empty((T, D), dtype=np.float32)
    for c in range(E):
        tc_ = toks[c]
        o = res.results[c]["outT"]  # [128, NCH, cap]
        out[tc_] = o.transpose(1, 0, 2).reshape(D, cap)[:, : len(tc_)].T
    return out


if __name__ == "__main__":
    print("smoke build only")
    _build_nc(312)
    print("built ok")


# revision 20
# speedup vs baseline: 1.0127x; 1.0127x over previous
"""LorentzMoE (top-1 routing, E=8 experts) on 8 Trainium2 NeuronCores.

Strategy (expert-parallel, host control plane):
  - Host computes the gate (softmax + bias + top-1) in numpy, exactly
    mirroring the reference numerics.
  - Tokens are dispatched by expert: core c gets every token routed to
    expert c (padded to a uniform capacity so one SPMD NEFF serves all
    8 cores).  Core c also computes the shared expert for those same
    tokens, the LResNet combine and the Lorentz normalization, so each
    token's full output is produced on a single core.
  - The gate weight is folded into the expert FFN: the W3 matmul
    consumes x pre-scaled by (2*w_tok) on the host, making the routed
    space output 2*w*o directly; the routed time component is
    sqrt(colsum(o'^2) + (2w)^2) which only needs a per-token row.
  - Host scatters per-core outputs back to the original token order.

Device layout: feature-on-partition ("transposed") everywhere.
  Weights are packed host-side as [128, 8, 1024] (partition-major) and
  DMAed as four quarter-matrices spread over four HWDGE queues
  (sync/gpsimd/scalar/vector) so packets are 4KB-contiguous per
  partition and aggregate DMA bandwidth is available from t=0.
  Column sums (over partitions) use ones-vector matmuls accumulated
  across the 8 chunks in PSUM; they are deferred out of the up-proj
  phases so PSUM stays shallow, and the |comb|^2 sum is folded into
  the shared-expert down-proj loop to avoid a serial epilogue.
"""

import numpy as np

D = 1024
E = 8
NCH = 8  # 1024 / 128 partition chunks
SCALE = 2.0

_cache: dict = {}


def _round_up(v, m):
    return ((v + m - 1) // m) * m


def _host_gate(x, gate_w, gate_b):
    """Replicates the reference gate in f32 numpy (verified bit-identical
    top-1 selection vs the jax reference on the benchmark inputs)."""
    logits = (x[:, 1:] @ gate_w.T).astype(np.float32)
    m = logits.max(-1, keepdims=True)
    e = np.exp(logits - m, dtype=np.float32)
    scores = e / e.sum(-1, keepdims=True, dtype=np.float32)
    biased = scores + gate_b
    idx = np.argmax(biased, axis=-1)
    w = scores[np.arange(x.shape[0]), idx]
    return idx.astype(np.int64), w.astype(np.float32)


def _build_nc(cap):
    import concourse.mybir as mybir
    import concourse.tile as tile
    from concourse import bacc

    f32 = mybir.dt.float32
    bf16 = mybir.dt.bfloat16
    AF = mybir.ActivationFunctionType
    ALU = mybir.AluOpType

    nc = bacc.Bacc("TRN2", target_bir_lowering=False, debug=False)

    # ---- DRAM I/O (weights packed [128, NCH, 1024]; x [128, NCH, cap]) ----
    xw_d = nc.dram_tensor("xw", [128, NCH, cap], bf16, kind="ExternalInput")
    xs_d = nc.dram_tensor("xs", [128, NCH, cap], bf16, kind="ExternalInput")
    w1_d = nc.dram_tensor("w1t", [128, NCH, D], bf16, kind="ExternalInput")
    w3_d = nc.dram_tensor("w3t", [128, NCH, D], bf16, kind="ExternalInput")
    w2_d = nc.dram_tensor("w2t", [128, NCH, D], bf16, kind="ExternalInput")
    v1_d = nc.dram_tensor("v1t", [128, NCH, D], bf16, kind="ExternalInput")
    v3_d = nc.dram_tensor("v3t", [128, NCH, D], bf16, kind="ExternalInput")
    v2_d = nc.dram_tensor("v2t", [128, NCH, D], bf16, kind="ExternalInput")
    r4_d = nc.dram_tensor("r4w2", [128, cap], f32, kind="ExternalInput")
    out_d = nc.dram_tensor("outT", [128, NCH, cap], f32, kind="ExternalOutput")

    with tile.TileContext(nc) as tc:
        with (
            tc.tile_pool(name="consts", bufs=1) as consts,
            tc.tile_pool(name="xpool", bufs=1) as xpool,
            tc.tile_pool(name="wpool", bufs=4) as wpool,
            tc.tile_pool(name="hpool", bufs=1) as hpool,
            tc.tile_pool(name="work", bufs=3) as work,
            tc.tile_pool(name="rows", bufs=1) as rows,
            tc.tile_pool(name="psum", bufs=8, space="PSUM") as psum,
        ):
            QS = [nc.sync, nc.gpsimd, nc.scalar]

            ones_mat = consts.tile([128, 128], f32)
            nc.vector.memset(ones_mat, 1.0)

            # warm up the PE HAM clock-gate with dummy matmuls while the
            # first weight DMAs are in flight (PE would be idle anyway)
            warm_ps = psum.tile([128, 128], f32, name="warm_ps", tag="mm")
            for _ in range(56):
                nc.tensor.matmul(
                    warm_ps, ones_mat, ones_mat, skip_group_check=True
                )

            # x first on each queue so the first matmuls have operands
            xw_a = xpool.tile([128, 4, cap], bf16)
            QS[0].dma_start(out=xw_a, in_=xw_d[:, 0:4, :])
            xw_b = xpool.tile([128, 4, cap], bf16)
            QS[1].dma_start(out=xw_b, in_=xw_d[:, 4:8, :])
            xs_a = xpool.tile([128, 4, cap], bf16)
            QS[2].dma_start(out=xs_a, in_=xs_d[:, 0:4, :])
            xs_b = xpool.tile([128, 4, cap], bf16)
            QS[2].dma_start(out=xs_b, in_=xs_d[:, 4:8, :])
            xw_sb = [(xw_a if k < 4 else xw_b)[:, k % 4, :] for k in range(NCH)]
            xs_sb = [(xs_a if k < 4 else xs_b)[:, k % 4, :] for k in range(NCH)]



            _w_count = [0]

            def load_w(dram, nm):
                """Four quarter-matrix tiles spread over the HWDGE queues."""
                qt = []
                off = _w_count[0]
                _w_count[0] += 1
                for q in range(4):
                    t = wpool.tile(
                        [128, 2, D], bf16, name=f"{nm}q{q}", tag=f"wq{q}"
                    )
                    QS[(q + off) % 3].dma_start(
                        out=t, in_=dram[:, 2 * q : 2 * q + 2, :]
                    )
                    qt.append(t)

                def sl(k, m):
                    return qt[k // 2][:, k % 2, 128 * m : 128 * (m + 1)]

                return sl

            # persistent activations
            h_r = [
                hpool.tile([128, cap], bf16, name=f"hr{k}", tag=f"hr{k}")
                for k in range(NCH)
            ]
            h_s = [
                hpool.tile([128, cap], bf16, name=f"hs{k}", tag=f"hs{k}")
                for k in range(NCH)
            ]
            oc = hpool.tile([128, NCH, cap], f32)

            def up_proj(wa, wb, xa, xb, h_out, interleave=False):
                """h_out[m] = bf16( silu(wa.T@xa) * (wb.T@xb) ) per chunk m.
                interleave=True orders the k-loops by DMA quarter arrival
                so the first phase can start before all weights landed."""
                for m in range(NCH):
                    ps1 = psum.tile([128, cap], f32, name=f"ps1_{m}", tag="mm")
                    ps3 = psum.tile([128, cap], f32, name=f"ps3_{m}", tag="mm")
                    if interleave:
                        for k in range(4):
                            nc.tensor.matmul(
                                ps1, wa(k, m), xa[k], start=(k == 0), stop=False,
                                skip_group_check=True,
                            )
                        for k in range(4):
                            nc.tensor.matmul(
                                ps3, wb(k, m), xb[k], start=(k == 0), stop=False,
                                skip_group_check=True,
                            )
                        for k in range(4, NCH):
                            nc.tensor.matmul(
                                ps1, wa(k, m), xa[k], start=False,
                                stop=(k == NCH - 1), skip_group_check=True,
                            )
                        for k in range(4, NCH):
                            nc.tensor.matmul(
                                ps3, wb(k, m), xb[k], start=False,
                                stop=(k == NCH - 1), skip_group_check=True,
                            )
                    else:
                        for k in range(NCH):
                            nc.tensor.matmul(
                                ps1, wa(k, m), xa[k], start=(k == 0),
                                stop=(k == NCH - 1),
                            )
                        for k in range(NCH):
                            nc.tensor.matmul(
                                ps3, wb(k, m), xb[k], start=(k == 0),
                                stop=(k == NCH - 1),
                            )
                    m1 = work.tile([128, cap], f32, name=f"m1_{m}", tag="m1")
                    nc.scalar.activation(m1, ps1, AF.Silu)
                    nc.vector.tensor_mul(h_out[m], m1, ps3)

            def colsum_sq(h_in, ps_t):
                """ps_t [128,cap] = colsum(h_in^2) replicated on all parts."""
                for m in range(NCH):
                    sq = work.tile([128, cap], f32, name=f"sq_{m}", tag="sq")
                    nc.vector.tensor_mul(sq, h_in[m], h_in[m])
                    nc.tensor.matmul(
                        ps_t,
                        ones_mat,
                        sq,
                        start=(m == 0),
                        stop=(m == NCH - 1),
                        skip_group_check=True,
                    )

            def down_proj(w2, h_in, ps_o, ps_li=None):
                """oc[:,m,:] (+)= w2.T @ h_in ; ps_o = colsum(o^2) replicated;
                if ps_li given (shared pass): oc += o and ps_li = colsum(oc^2)."""
                for m in range(NCH):
                    po = psum.tile([128, cap], f32, name=f"po_{m}", tag="mm")
                    ks = list(range(1, NCH)) + [0]
                    for j, k in enumerate(ks):
                        nc.tensor.matmul(
                            po, w2(k, m), h_in[k], start=(j == 0), stop=(j == NCH - 1)
                        )
                    if ps_li is not None:
                        sq = work.tile([128, cap], f32, name=f"sqo_{m}", tag="sq")
                        nc.scalar.square(sq, po)
                        nc.vector.tensor_add(oc[:, m, :], oc[:, m, :], po)
                        sqc = work.tile([128, cap], f32, name=f"sqc_{m}", tag="sqc")
                        nc.vector.tensor_mul(sqc, oc[:, m, :], oc[:, m, :])
                    else:
                        nc.scalar.copy(oc[:, m, :], po)
                        sq = work.tile([128, cap], f32, name=f"sqo_{m}", tag="sq")
                        nc.vector.tensor_mul(sq, oc[:, m, :], oc[:, m, :])
                    nc.tensor.matmul(
                        ps_o,
                        ones_mat,
                        sq,
                        start=(m == 0),
                        stop=(m == NCH - 1),
                        skip_group_check=True,
                    )
                    if ps_li is not None:
                        nc.tensor.matmul(
                            ps_li,
                            ones_mat,
                            sqc,
                            start=(m == 0),
                            stop=(m == NCH - 1),
                            skip_group_check=True,
                        )

            # ================= routed expert =================
            w1 = load_w(w1_d, "w1")
            w3 = load_w(w3_d, "w3")
            up_proj(w1, w3, xw_sb, xs_sb, h_r, interleave=True)
            w2 = load_w(w2_d, "w2")
            # r4 load is only needed mid-kernel; keep it off the early queues
            r4_sb = rows.tile([128, cap], f32)
            QS[2].dma_start(out=r4_sb, in_=r4_d[:, :])
            # t' = sqrt(colsum(sp'^2) + (2w)^2) -> bf16 row 0 of h_r[0]
            ps_tr = psum.tile([128, cap], f32, name="ps_tr", tag="mm")
            colsum_sq(h_r, ps_tr)
            trow = rows.tile([1, cap], f32)
            nc.vector.tensor_add(trow, ps_tr[0:1, :], r4_sb[0:1, :])
            nc.scalar.activation(h_r[0][0:1, :], trow, AF.Sqrt)

            ps_or = psum.tile([128, cap], f32, name="ps_or", tag="mm")
            down_proj(w2, h_r, ps_or)

            # ================= shared expert =================
            v1 = load_w(v1_d, "v1")
            v3 = load_w(v3_d, "v3")
            up_proj(v1, v3, xw_sb, xw_sb, h_s)
            v2 = load_w(v2_d, "v2")
            ps_ts = psum.tile([128, cap], f32, name="ps_ts", tag="mm")
            colsum_sq(h_s, ps_ts)
            nc.scalar.activation(h_s[0][0:1, :], ps_ts[0:1, :], AF.Sqrt, bias=1.0)

            ps_os = psum.tile([128, cap], f32, name="ps_os", tag="mm")
            ps_li = psum.tile([128, cap], f32, name="ps_li", tag="mm")
            down_proj(v2, h_s, ps_os, ps_li=ps_li)

            # ================= combine + Lorentz normalize =================
            # (all row quantities replicated across 128 partitions; all
            #  sqrt/rsqrt in the tail via the Abs_reciprocal_sqrt table so
            #  no activation-table reload lands on the critical chain)
            # comb_space = o_shared + 2w*o_routed  (already summed in oc)
            # comb_time  = ot_shared + 2 + 2w*ot_routed
            a_or = rows.tile([128, cap], f32)
            nc.vector.tensor_add(a_or, ps_or, r4_sb)
            r_or = rows.tile([128, cap], f32)
            nc.scalar.activation(r_or, a_or, AF.Abs_reciprocal_sqrt)
            orow = rows.tile([128, cap], f32)
            nc.vector.tensor_mul(orow, a_or, r_or)  # = 2w*ot_routed
            a_os = rows.tile([128, cap], f32)
            nc.vector.tensor_scalar_add(a_os, ps_os, 1.0)
            r_os = rows.tile([128, cap], f32)
            nc.scalar.activation(r_os, a_os, AF.Abs_reciprocal_sqrt)
            osrow = rows.tile([128, cap], f32)
            nc.vector.tensor_mul(osrow, a_os, r_os)  # ot_shared
            ct = rows.tile([128, cap], f32)
            nc.vector.scalar_tensor_tensor(
                ct, orow, 2.0, osrow, op0=ALU.add, op1=ALU.add
            )
            ct2 = rows.tile([128, cap], f32)
            nc.vector.tensor_mul(ct2, ct, ct)
            absli = rows.tile([128, cap], f32)
            nc.vector.tensor_sub(absli, ct2, ps_li)  # = |<comb,comb>_L| > 0
            inv = rows.tile([128, cap], f32)
            nc.scalar.activation(inv, absli, AF.Abs_reciprocal_sqrt)
            otime = rows.tile([1, cap], f32)
            nc.vector.tensor_mul(otime, ct[0:1, :], inv[0:1, :])

            # batched scale by inv (free-dim broadcast), quartered so the
            # output DMAs overlap the remaining scales
            import concourse.bass as bass

            inv_b2 = bass.AP(
                tensor=inv.tensor,
                offset=inv.offset,
                ap=[inv.ap[0], [0, 2], inv.ap[1]],
            )
            for q in range(4):
                sl = slice(2 * q, 2 * q + 2)
                nc.vector.tensor_mul(oc[:, sl, :], oc[:, sl, :], inv_b2)
                if q == 0:
                    nc.vector.tensor_copy(oc[0:1, 0, :], otime)
                QS[q % 3].dma_start(out=out_d[:, sl, :], in_=oc[:, sl, :])

    nc.compile()
    return nc


def _get_nc(cap):
    if cap not in _cache:
        _cache[cap] = _build_nc(cap)
    return _cache[cap]


def _pack_w(mat_t):
    """[1024, 1024] (K, M) -> [128, 8, 1024] partition-major bf16."""
    return np.ascontiguousarray(mat_t.reshape(NCH, 128, D).transpose(1, 0, 2))


def kernel(x, gate_w, gate_b, W1, W3, W2, Ws1, Ws3, Ws2):
    import ml_dtypes

    from concourse.bass_utils import run_bass_kernel_spmd

    bf16 = ml_dtypes.bfloat16
    x = np.asarray(x, dtype=np.float32)
    gate_w = np.asarray(gate_w, dtype=np.float32)
    gate_b = np.asarray(gate_b, dtype=np.float32)
    W1 = np.asarray(W1, dtype=np.float32)
    W3 = np.asarray(W3, dtype=np.float32)
    W2 = np.asarray(W2, dtype=np.float32)
    Ws1 = np.asarray(Ws1, dtype=np.float32)
    Ws3 = np.asarray(Ws3, dtype=np.float32)
    Ws2 = np.asarray(Ws2, dtype=np.float32)

    T = x.shape[0]
    idx, w = _host_gate(x, gate_w, gate_b)
    counts = np.bincount(idx, minlength=E)
    cap = max(64, _round_up(int(counts.max()), 8))
    assert cap <= 512, f"capacity {cap} exceeds single-block limit"

    toks = [np.where(idx == c)[0] for c in range(E)]

    # host-side weight prep: transposed [K, M] with zero col for the time
    # slot, then packed partition-major
    def prep(Wm):  # [A, B] -> [B, A+1] bf16 (col 0 zero), packed
        out = np.zeros((D, D), dtype=bf16)
        out[:, 1:] = Wm.T.astype(bf16)
        return _pack_w(out)

    v1t = prep(Ws1)
    v3t = prep(Ws3)
    v2t = prep(Ws2)

    in_maps = []
    for c in range(E):
        tc_ = toks[c]
        n = len(tc_)
        xt = np.zeros((NCH, 128, cap), dtype=bf16)
        xt.reshape(D, cap)[:, :n] = x[tc_].T.astype(bf16)
        xs = np.zeros((NCH, 128, cap), dtype=bf16)
        tw = (SCALE * w[tc_]).astype(np.float32)
        xs.reshape(D, cap)[:, :n] = (x[tc_] * tw[:, None]).T.astype(bf16)
        r4row = np.zeros((cap,), dtype=np.float32)
        r4row[:n] = tw * tw
        r4 = np.ascontiguousarray(np.broadcast_to(r4row, (128, cap)))
        in_maps.append(
            {
                "xw": np.ascontiguousarray(xt.transpose(1, 0, 2)),
                "xs": np.ascontiguousarray(xs.transpose(1, 0, 2)),
                "w1t": prep(W1[c]),
                "w3t": prep(W3[c]),
                "w2t": prep(W2[c]),
                "v1t": v1t,
                "v3t": v3t,
                "v2t": v2t,
                "r4w2": r4,
            }
        )

    nc = _get_nc(cap)
    res = run_bass_kernel_spmd(nc, in_maps, core_ids=list(range(E)))

    out = np.empty((T, D), dtype=np.float32)
    for c in range(E):
        tc_ = toks[c]
        o = res.results[c]["outT"]  # [128, NCH, cap]
        out[tc_] = o.transpose(1, 0, 2).reshape(D, cap)[:, : len(tc_)].T
    return out


if __name__ == "__main__":
    print("smoke build only")
    _build_nc(312)
    print("built ok")


# revision 25
# speedup vs baseline: 1.0944x; 1.0807x over previous
"""LorentzMoE (top-1 routing, E=8 experts) on 8 Trainium2 NeuronCores.

Strategy (expert-parallel, host control plane):
  - Host computes the gate (softmax + bias + top-1) in numpy, exactly
    mirroring the reference numerics.
  - Tokens are dispatched by expert: core c gets every token routed to
    expert c (padded to a uniform capacity so one SPMD NEFF serves all
    8 cores).  Core c also computes the shared expert for those same
    tokens, the LResNet combine and the Lorentz normalization, so each
    token's full output is produced on a single core.
  - The gate weight is folded into the expert FFN: the W3 matmul
    consumes x pre-scaled by (2*w_tok) on the host, making the routed
    space output 2*w*o directly; the routed time component is
    sqrt(colsum(o'^2) + (2w)^2) which only needs a per-token row.
  - Host scatters per-core outputs back to the original token order.

Device layout: feature-on-partition ("transposed") everywhere.
  Weights are packed host-side as [128, 8, 1024] (partition-major) and
  DMAed as four quarter-matrices spread over four HWDGE queues
  (sync/gpsimd/scalar/vector) so packets are 4KB-contiguous per
  partition and aggregate DMA bandwidth is available from t=0.
  Column sums (over partitions) use ones-vector matmuls accumulated
  across the 8 chunks in PSUM; they are deferred out of the up-proj
  phases so PSUM stays shallow, and the |comb|^2 sum is folded into
  the shared-expert down-proj loop to avoid a serial epilogue.
"""

import numpy as np

D = 1024
E = 8
NCH = 8  # 1024 / 128 partition chunks
SCALE = 2.0
UP_FP8 = True  # fp8e4m3 weights for the four up-projection matrices

_cache: dict = {}


def _round_up(v, m):
    return ((v + m - 1) // m) * m


def _host_gate(x, gate_w, gate_b):
    """Replicates the reference gate in f32 numpy (verified bit-identical
    top-1 selection vs the jax reference on the benchmark inputs)."""
    logits = (x[:, 1:] @ gate_w.T).astype(np.float32)
    m = logits.max(-1, keepdims=True)
    e = np.exp(logits - m, dtype=np.float32)
    scores = e / e.sum(-1, keepdims=True, dtype=np.float32)
    biased = scores + gate_b
    idx = np.argmax(biased, axis=-1)
    w = scores[np.arange(x.shape[0]), idx]
    return idx.astype(np.int64), w.astype(np.float32)


def _build_nc(cap):
    import concourse.mybir as mybir
    import concourse.tile as tile
    from concourse import bacc

    f32 = mybir.dt.float32
    bf16 = mybir.dt.bfloat16
    AF = mybir.ActivationFunctionType
    ALU = mybir.AluOpType

    nc = bacc.Bacc("TRN2", target_bir_lowering=False, debug=False)

    f8 = mybir.dt.float8e4
    up_dt = f8 if UP_FP8 else bf16

    # ---- DRAM I/O (weights packed [128, NCH, 1024]; x [128, NCH, cap]) ----
    xw_d = nc.dram_tensor("xw", [128, NCH, cap], bf16, kind="ExternalInput")
    xs_d = nc.dram_tensor("xs", [128, NCH, cap], bf16, kind="ExternalInput")
    w1_d = nc.dram_tensor("w1t", [128, NCH, D], up_dt, kind="ExternalInput")
    w3_d = nc.dram_tensor("w3t", [128, NCH, D], up_dt, kind="ExternalInput")
    w2_d = nc.dram_tensor("w2t", [128, NCH, D], bf16, kind="ExternalInput")
    v1_d = nc.dram_tensor("v1t", [128, NCH, D], up_dt, kind="ExternalInput")
    v3_d = nc.dram_tensor("v3t", [128, NCH, D], up_dt, kind="ExternalInput")
    v2_d = nc.dram_tensor("v2t", [128, NCH, D], bf16, kind="ExternalInput")
    r4_d = nc.dram_tensor("r4w2", [128, cap], f32, kind="ExternalInput")
    out_d = nc.dram_tensor("outT", [128, NCH, cap], f32, kind="ExternalOutput")

    with tile.TileContext(nc) as tc:
        with (
            tc.tile_pool(name="consts", bufs=1) as consts,
            tc.tile_pool(name="xpool", bufs=1) as xpool,
            tc.tile_pool(name="wpool", bufs=4) as wpool,
            tc.tile_pool(name="hpool", bufs=1) as hpool,
            tc.tile_pool(name="work", bufs=3) as work,
            tc.tile_pool(name="rows", bufs=1) as rows,
            tc.tile_pool(name="psum", bufs=8, space="PSUM") as psum,
        ):
            QS = [nc.sync, nc.gpsimd, nc.scalar]

            ones_mat = consts.tile([128, 128], f32)
            nc.vector.memset(ones_mat, 1.0)

            # warm up the PE HAM clock-gate with dummy matmuls while the
            # first weight DMAs are in flight (PE would be idle anyway)
            warm_ps = psum.tile([128, 128], f32, name="warm_ps", tag="mm")
            for _ in range(56):
                nc.tensor.matmul(
                    warm_ps, ones_mat, ones_mat, skip_group_check=True
                )

            # x first on each queue so the first matmuls have operands
            xw_a = xpool.tile([128, 4, cap], bf16)
            QS[0].dma_start(out=xw_a, in_=xw_d[:, 0:4, :])
            xw_b = xpool.tile([128, 4, cap], bf16)
            QS[1].dma_start(out=xw_b, in_=xw_d[:, 4:8, :])
            xs_a = xpool.tile([128, 4, cap], bf16)
            QS[2].dma_start(out=xs_a, in_=xs_d[:, 0:4, :])
            xs_b = xpool.tile([128, 4, cap], bf16)
            QS[2].dma_start(out=xs_b, in_=xs_d[:, 4:8, :])
            xw_sb = [(xw_a if k < 4 else xw_b)[:, k % 4, :] for k in range(NCH)]
            xs_sb = [(xs_a if k < 4 else xs_b)[:, k % 4, :] for k in range(NCH)]



            _w_count = [0]

            def load_w(dram, nm):
                """Four quarter-matrix tiles spread over the HWDGE queues."""
                qt = []
                off = _w_count[0]
                _w_count[0] += 1
                for q in range(4):
                    t = wpool.tile(
                        [128, 2, D], dram.dtype, name=f"{nm}q{q}", tag=f"wq{q}"
                    )
                    QS[(q + off) % 3].dma_start(
                        out=t, in_=dram[:, 2 * q : 2 * q + 2, :]
                    )
                    qt.append(t)

                def sl(k, m):
                    return qt[k // 2][:, k % 2, 128 * m : 128 * (m + 1)]

                return sl

            # persistent activations
            h_r = [
                hpool.tile([128, cap], bf16, name=f"hr{k}", tag=f"hr{k}")
                for k in range(NCH)
            ]
            h_s = [
                hpool.tile([128, cap], bf16, name=f"hs{k}", tag=f"hs{k}")
                for k in range(NCH)
            ]
            oc = hpool.tile([128, NCH, cap], f32)

            def up_proj(wa, wb, xa, xb, h_out, interleave=False):
                """h_out[m] = bf16( silu(wa.T@xa) * (wb.T@xb) ) per chunk m.
                interleave=True orders the k-loops by DMA quarter arrival
                so the first phase can start before all weights landed."""
                for m in range(NCH):
                    ps1 = psum.tile([128, cap], f32, name=f"ps1_{m}", tag="mm")
                    ps3 = psum.tile([128, cap], f32, name=f"ps3_{m}", tag="mm")
                    if interleave:
                        for k in range(4):
                            nc.tensor.matmul(
                                ps1, wa(k, m), xa[k], start=(k == 0), stop=False,
                                skip_group_check=True,
                            )
                        for k in range(4):
                            nc.tensor.matmul(
                                ps3, wb(k, m), xb[k], start=(k == 0), stop=False,
                                skip_group_check=True,
                            )
                        for k in range(4, NCH):
                            nc.tensor.matmul(
                                ps1, wa(k, m), xa[k], start=False,
                                stop=(k == NCH - 1), skip_group_check=True,
                            )
                        for k in range(4, NCH):
                            nc.tensor.matmul(
                                ps3, wb(k, m), xb[k], start=False,
                                stop=(k == NCH - 1), skip_group_check=True,
                            )
                    else:
                        for k in range(NCH):
                            nc.tensor.matmul(
                                ps1, wa(k, m), xa[k], start=(k == 0),
                                stop=(k == NCH - 1),
                            )
                        for k in range(NCH):
                            nc.tensor.matmul(
                                ps3, wb(k, m), xb[k], start=(k == 0),
                                stop=(k == NCH - 1),
                            )
                    m1 = work.tile([128, cap], f32, name=f"m1_{m}", tag="m1")
                    nc.scalar.activation(m1, ps1, AF.Silu)
                    nc.vector.tensor_mul(h_out[m], m1, ps3)

            def colsum_sq(h_in, ps_t):
                """ps_t [128,cap] = colsum(h_in^2) replicated on all parts."""
                for m in range(NCH):
                    sq = work.tile([128, cap], f32, name=f"sq_{m}", tag="sq")
                    nc.vector.tensor_mul(sq, h_in[m], h_in[m])
                    nc.tensor.matmul(
                        ps_t,
                        ones_mat,
                        sq,
                        start=(m == 0),
                        stop=(m == NCH - 1),
                        skip_group_check=True,
                    )

            def down_proj(w2, h_in, ps_o, ps_li=None):
                """oc[:,m,:] (+)= w2.T @ h_in ; ps_o = colsum(o^2) replicated;
                if ps_li given (shared pass): oc += o and ps_li = colsum(oc^2)."""
                for m in range(NCH):
                    po = psum.tile([128, cap], f32, name=f"po_{m}", tag="mm")
                    ks = list(range(1, NCH)) + [0]
                    for j, k in enumerate(ks):
                        nc.tensor.matmul(
                            po, w2(k, m), h_in[k], start=(j == 0), stop=(j == NCH - 1)
                        )
                    if ps_li is not None:
                        sq = work.tile([128, cap], f32, name=f"sqo_{m}", tag="sq")
                        nc.scalar.square(sq, po)
                        nc.vector.tensor_add(oc[:, m, :], oc[:, m, :], po)
                        sqc = work.tile([128, cap], f32, name=f"sqc_{m}", tag="sqc")
                        nc.vector.tensor_mul(sqc, oc[:, m, :], oc[:, m, :])
                    else:
                        nc.scalar.copy(oc[:, m, :], po)
                        sq = work.tile([128, cap], f32, name=f"sqo_{m}", tag="sq")
                        nc.vector.tensor_mul(sq, oc[:, m, :], oc[:, m, :])
                    nc.tensor.matmul(
                        ps_o,
                        ones_mat,
                        sq,
                        start=(m == 0),
                        stop=(m == NCH - 1),
                        skip_group_check=True,
                    )
                    if ps_li is not None:
                        nc.tensor.matmul(
                            ps_li,
                            ones_mat,
                            sqc,
                            start=(m == 0),
                            stop=(m == NCH - 1),
                            skip_group_check=True,
                        )

            # ================= routed expert =================
            w1 = load_w(w1_d, "w1")
            w3 = load_w(w3_d, "w3")
            up_proj(w1, w3, xw_sb, xs_sb, h_r, interleave=True)
            w2 = load_w(w2_d, "w2")
            # r4 load is only needed mid-kernel; keep it off the early queues
            r4_sb = rows.tile([128, cap], f32)
            QS[2].dma_start(out=r4_sb, in_=r4_d[:, :])
            # t' = sqrt(colsum(sp'^2) + (2w)^2) -> bf16 row 0 of h_r[0]
            ps_tr = psum.tile([128, cap], f32, name="ps_tr", tag="mm")
            colsum_sq(h_r, ps_tr)
            trow = rows.tile([1, cap], f32)
            nc.vector.tensor_add(trow, ps_tr[0:1, :], r4_sb[0:1, :])
            nc.scalar.activation(h_r[0][0:1, :], trow, AF.Sqrt)

            ps_or = psum.tile([128, cap], f32, name="ps_or", tag="mm")
            down_proj(w2, h_r, ps_or)

            # ================= shared expert =================
            v1 = load_w(v1_d, "v1")
            v3 = load_w(v3_d, "v3")
            up_proj(v1, v3, xw_sb, xw_sb, h_s)
            v2 = load_w(v2_d, "v2")
            ps_ts = psum.tile([128, cap], f32, name="ps_ts", tag="mm")
            colsum_sq(h_s, ps_ts)
            nc.scalar.activation(h_s[0][0:1, :], ps_ts[0:1, :], AF.Sqrt, bias=1.0)

            ps_os = psum.tile([128, cap], f32, name="ps_os", tag="mm")
            ps_li = psum.tile([128, cap], f32, name="ps_li", tag="mm")
            down_proj(v2, h_s, ps_os, ps_li=ps_li)

            # ================= combine + Lorentz normalize =================
            # (all row quantities replicated across 128 partitions; all
            #  sqrt/rsqrt in the tail via the Abs_reciprocal_sqrt table so
            #  no activation-table reload lands on the critical chain)
            # comb_space = o_shared + 2w*o_routed  (already summed in oc)
            # comb_time  = ot_shared + 2 + 2w*ot_routed
            a_or = rows.tile([128, cap], f32)
            nc.vector.tensor_add(a_or, ps_or, r4_sb)
            r_or = rows.tile([128, cap], f32)
            nc.scalar.activation(r_or, a_or, AF.Abs_reciprocal_sqrt)
            orow = rows.tile([128, cap], f32)
            nc.vector.tensor_mul(orow, a_or, r_or)  # = 2w*ot_routed
            a_os = rows.tile([128, cap], f32)
            nc.vector.tensor_scalar_add(a_os, ps_os, 1.0)
            r_os = rows.tile([128, cap], f32)
            nc.scalar.activation(r_os, a_os, AF.Abs_reciprocal_sqrt)
            osrow = rows.tile([128, cap], f32)
            nc.vector.tensor_mul(osrow, a_os, r_os)  # ot_shared
            ct = rows.tile([128, cap], f32)
            nc.vector.scalar_tensor_tensor(
                ct, orow, 2.0, osrow, op0=ALU.add, op1=ALU.add
            )
            ct2 = rows.tile([128, cap], f32)
            nc.vector.tensor_mul(ct2, ct, ct)
            absli = rows.tile([128, cap], f32)
            nc.vector.tensor_sub(absli, ct2, ps_li)  # = |<comb,comb>_L| > 0
            inv = rows.tile([128, cap], f32)
            nc.scalar.activation(inv, absli, AF.Abs_reciprocal_sqrt)
            otime = rows.tile([1, cap], f32)
            nc.vector.tensor_mul(otime, ct[0:1, :], inv[0:1, :])

            # batched scale by inv (free-dim broadcast), quartered so the
            # output DMAs overlap the remaining scales
            import concourse.bass as bass

            inv_b2 = bass.AP(
                tensor=inv.tensor,
                offset=inv.offset,
                ap=[inv.ap[0], [0, 2], inv.ap[1]],
            )
            for q in range(4):
                sl = slice(2 * q, 2 * q + 2)
                nc.vector.tensor_mul(oc[:, sl, :], oc[:, sl, :], inv_b2)
                if q == 0:
                    nc.vector.tensor_copy(oc[0:1, 0, :], otime)
                QS[q % 3].dma_start(out=out_d[:, sl, :], in_=oc[:, sl, :])

    nc.compile()
    return nc


def _get_nc(cap):
    if cap not in _cache:
        _cache[cap] = _build_nc(cap)
    return _cache[cap]


def _pack_w(mat_t):
    """[1024, 1024] (K, M) -> [128, 8, 1024] partition-major bf16."""
    return np.ascontiguousarray(mat_t.reshape(NCH, 128, D).transpose(1, 0, 2))


def kernel(x, gate_w, gate_b, W1, W3, W2, Ws1, Ws3, Ws2):
    import ml_dtypes

    from concourse.bass_utils import run_bass_kernel_spmd

    bf16 = ml_dtypes.bfloat16
    x = np.asarray(x, dtype=np.float32)
    gate_w = np.asarray(gate_w, dtype=np.float32)
    gate_b = np.asarray(gate_b, dtype=np.float32)
    W1 = np.asarray(W1, dtype=np.float32)
    W3 = np.asarray(W3, dtype=np.float32)
    W2 = np.asarray(W2, dtype=np.float32)
    Ws1 = np.asarray(Ws1, dtype=np.float32)
    Ws3 = np.asarray(Ws3, dtype=np.float32)
    Ws2 = np.asarray(Ws2, dtype=np.float32)

    T = x.shape[0]
    idx, w = _host_gate(x, gate_w, gate_b)
    counts = np.bincount(idx, minlength=E)
    cap = max(64, _round_up(int(counts.max()), 8))
    assert cap <= 512, f"capacity {cap} exceeds single-block limit"

    toks = [np.where(idx == c)[0] for c in range(E)]

    f8 = ml_dtypes.float8_e4m3
    up_dt = f8 if UP_FP8 else bf16

    # host-side weight prep: transposed [K, M] with zero col for the time
    # slot, then packed partition-major
    def prep(Wm, dt=bf16):  # [A, B] -> [B, A+1] (col 0 zero), packed
        out = np.zeros((D, D), dtype=dt)
        out[:, 1:] = Wm.T.astype(dt)
        return _pack_w(out)

    v1t = prep(Ws1, up_dt)
    v3t = prep(Ws3, up_dt)
    v2t = prep(Ws2)

    in_maps = []
    for c in range(E):
        tc_ = toks[c]
        n = len(tc_)
        xt = np.zeros((NCH, 128, cap), dtype=bf16)
        xt.reshape(D, cap)[:, :n] = x[tc_].T.astype(bf16)
        xs = np.zeros((NCH, 128, cap), dtype=bf16)
        tw = (SCALE * w[tc_]).astype(np.float32)
        xs.reshape(D, cap)[:, :n] = (x[tc_] * tw[:, None]).T.astype(bf16)
        r4row = np.zeros((cap,), dtype=np.float32)
        r4row[:n] = tw * tw
        r4 = np.ascontiguousarray(np.broadcast_to(r4row, (128, cap)))
        in_maps.append(
            {
                "xw": np.ascontiguousarray(xt.transpose(1, 0, 2)),
                "xs": np.ascontiguousarray(xs.transpose(1, 0, 2)),
                "w1t": prep(W1[c], up_dt),
                "w3t": prep(W3[c], up_dt),
                "w2t": prep(W2[c]),
                "v1t": v1t,
                "v3t": v3t,
                "v2t": v2t,
                "r4w2": r4,
            }
        )

    nc = _get_nc(cap)
    res = run_bass_kernel_spmd(nc, in_maps, core_ids=list(range(E)))

    out = np.empty((T, D), dtype=np.float32)
    for c in range(E):
        tc_ = toks[c]
        o = res.results[c]["outT"]  # [128, NCH, cap]
        out[tc_] = o.transpose(1, 0, 2).reshape(D, cap)[:, : len(tc_)].T
    return out


if __name__ == "__main__":
    print("smoke build only")
    _build_nc(312)
    print("built ok")


# revision 26
# speedup vs baseline: 1.2040x; 1.1001x over previous
"""LorentzMoE (top-1 routing, E=8 experts) on 8 Trainium2 NeuronCores.

Strategy (expert-parallel, host control plane):
  - Host computes the gate (softmax + bias + top-1) in numpy, exactly
    mirroring the reference numerics.
  - Tokens are dispatched by expert: core c gets every token routed to
    expert c (padded to a uniform capacity so one SPMD NEFF serves all
    8 cores).  Core c also computes the shared expert for those same
    tokens, the LResNet combine and the Lorentz normalization, so each
    token's full output is produced on a single core.
  - The gate weight is folded into the expert FFN: the W3 matmul
    consumes x pre-scaled by (2*w_tok) on the host, making the routed
    space output 2*w*o directly; the routed time component is
    sqrt(colsum(o'^2) + (2w)^2) which only needs a per-token row.
  - Host scatters per-core outputs back to the original token order.

Device layout: feature-on-partition ("transposed") everywhere.
  Weights are packed host-side as [128, 8, 1024] (partition-major) and
  DMAed as four quarter-matrices spread over four HWDGE queues
  (sync/gpsimd/scalar/vector) so packets are 4KB-contiguous per
  partition and aggregate DMA bandwidth is available from t=0.
  Column sums (over partitions) use ones-vector matmuls accumulated
  across the 8 chunks in PSUM; they are deferred out of the up-proj
  phases so PSUM stays shallow, and the |comb|^2 sum is folded into
  the shared-expert down-proj loop to avoid a serial epilogue.
"""

import numpy as np

D = 1024
E = 8
NCH = 8  # 1024 / 128 partition chunks
SCALE = 2.0
UP_FP8 = True  # fp8e4m3 weights for the four up-projection matrices

_cache: dict = {}


def _round_up(v, m):
    return ((v + m - 1) // m) * m


def _host_gate(x, gate_w, gate_b):
    """Replicates the reference gate in f32 numpy (verified bit-identical
    top-1 selection vs the jax reference on the benchmark inputs)."""
    logits = (x[:, 1:] @ gate_w.T).astype(np.float32)
    m = logits.max(-1, keepdims=True)
    e = np.exp(logits - m, dtype=np.float32)
    scores = e / e.sum(-1, keepdims=True, dtype=np.float32)
    biased = scores + gate_b
    idx = np.argmax(biased, axis=-1)
    w = scores[np.arange(x.shape[0]), idx]
    return idx.astype(np.int64), w.astype(np.float32)


def _build_nc(cap):
    import concourse.mybir as mybir
    import concourse.tile as tile
    from concourse import bacc

    f32 = mybir.dt.float32
    bf16 = mybir.dt.bfloat16
    AF = mybir.ActivationFunctionType
    ALU = mybir.AluOpType

    nc = bacc.Bacc("TRN2", target_bir_lowering=False, debug=False)

    f8 = mybir.dt.float8e4
    up_dt = f8 if UP_FP8 else bf16

    # ---- DRAM I/O (weights packed [128, NCH, 1024]; x [128, NCH, cap]) ----
    xw_d = nc.dram_tensor("xw", [128, NCH, cap], bf16, kind="ExternalInput")
    xs_d = nc.dram_tensor("xs", [128, NCH, cap], bf16, kind="ExternalInput")
    w1_d = nc.dram_tensor("w1t", [128, NCH, D], up_dt, kind="ExternalInput")
    w3_d = nc.dram_tensor("w3t", [128, NCH, D], up_dt, kind="ExternalInput")
    w2_d = nc.dram_tensor("w2t", [128, NCH, D], bf16, kind="ExternalInput")
    v1_d = nc.dram_tensor("v1t", [128, NCH, D], up_dt, kind="ExternalInput")
    v3_d = nc.dram_tensor("v3t", [128, NCH, D], up_dt, kind="ExternalInput")
    v2_d = nc.dram_tensor("v2t", [128, NCH, D], bf16, kind="ExternalInput")
    r4_d = nc.dram_tensor("r4w2", [128, cap], f32, kind="ExternalInput")
    out_d = nc.dram_tensor("outT", [128, NCH, cap], f32, kind="ExternalOutput")

    with tile.TileContext(nc) as tc:
        with (
            tc.tile_pool(name="consts", bufs=1) as consts,
            tc.tile_pool(name="xpool", bufs=1) as xpool,
            tc.tile_pool(name="wpool", bufs=4) as wpool,
            tc.tile_pool(name="hpool", bufs=1) as hpool,
            tc.tile_pool(name="work", bufs=3) as work,
            tc.tile_pool(name="rows", bufs=1) as rows,
            tc.tile_pool(name="psum", bufs=8, space="PSUM") as psum,
        ):
            QS = [nc.sync, nc.gpsimd, nc.scalar]

            ones_mat = consts.tile([128, 128], bf16)
            nc.vector.memset(ones_mat, 1.0)

            # warm up the PE HAM clock-gate with dummy matmuls while the
            # first weight DMAs are in flight (PE would be idle anyway)
            warm_ps = psum.tile([128, 128], f32, name="warm_ps", tag="mm")
            for _ in range(56):
                nc.tensor.matmul(
                    warm_ps, ones_mat, ones_mat, skip_group_check=True
                )

            # x first on each queue so the first matmuls have operands
            xw_a = xpool.tile([128, 4, cap], bf16)
            QS[0].dma_start(out=xw_a, in_=xw_d[:, 0:4, :])
            xw_b = xpool.tile([128, 4, cap], bf16)
            QS[1].dma_start(out=xw_b, in_=xw_d[:, 4:8, :])
            xs_a = xpool.tile([128, 4, cap], bf16)
            QS[2].dma_start(out=xs_a, in_=xs_d[:, 0:4, :])
            xs_b = xpool.tile([128, 4, cap], bf16)
            QS[2].dma_start(out=xs_b, in_=xs_d[:, 4:8, :])
            xw_sb = [(xw_a if k < 4 else xw_b)[:, k % 4, :] for k in range(NCH)]
            xs_sb = [(xs_a if k < 4 else xs_b)[:, k % 4, :] for k in range(NCH)]



            _w_count = [0]

            def load_w(dram, nm):
                """Four quarter-matrix tiles spread over the HWDGE queues."""
                qt = []
                off = _w_count[0]
                _w_count[0] += 1
                for q in range(4):
                    t = wpool.tile(
                        [128, 2, D], dram.dtype, name=f"{nm}q{q}", tag=f"wq{q}"
                    )
                    QS[(q + off) % 3].dma_start(
                        out=t, in_=dram[:, 2 * q : 2 * q + 2, :]
                    )
                    qt.append(t)

                def sl(k, m):
                    return qt[k // 2][:, k % 2, 128 * m : 128 * (m + 1)]

                return sl

            # persistent activations
            h_r = [
                hpool.tile([128, cap], bf16, name=f"hr{k}", tag=f"hr{k}")
                for k in range(NCH)
            ]
            h_s = [
                hpool.tile([128, cap], bf16, name=f"hs{k}", tag=f"hs{k}")
                for k in range(NCH)
            ]
            oc = hpool.tile([128, NCH, cap], f32)

            def up_proj(wa, wb, xa, xb, h_out, interleave=False):
                """h_out[m] = bf16( silu(wa.T@xa) * (wb.T@xb) ) per chunk m.
                interleave=True orders the k-loops by DMA quarter arrival
                so the first phase can start before all weights landed."""
                for m in range(NCH):
                    ps1 = psum.tile([128, cap], f32, name=f"ps1_{m}", tag="mm")
                    ps3 = psum.tile([128, cap], f32, name=f"ps3_{m}", tag="mm")
                    if interleave:
                        for k in range(4):
                            nc.tensor.matmul(
                                ps1, wa(k, m), xa[k], start=(k == 0), stop=False,
                                skip_group_check=True,
                            )
                        for k in range(4):
                            nc.tensor.matmul(
                                ps3, wb(k, m), xb[k], start=(k == 0), stop=False,
                                skip_group_check=True,
                            )
                        for k in range(4, NCH):
                            nc.tensor.matmul(
                                ps1, wa(k, m), xa[k], start=False,
                                stop=(k == NCH - 1), skip_group_check=True,
                            )
                        for k in range(4, NCH):
                            nc.tensor.matmul(
                                ps3, wb(k, m), xb[k], start=False,
                                stop=(k == NCH - 1), skip_group_check=True,
                            )
                    else:
                        for k in range(NCH):
                            nc.tensor.matmul(
                                ps1, wa(k, m), xa[k], start=(k == 0),
                                stop=(k == NCH - 1),
                            )
                        for k in range(NCH):
                            nc.tensor.matmul(
                                ps3, wb(k, m), xb[k], start=(k == 0),
                                stop=(k == NCH - 1),
                            )
                    m1 = work.tile([128, cap], f32, name=f"m1_{m}", tag="m1")
                    nc.scalar.activation(m1, ps1, AF.Silu)
                    nc.vector.tensor_mul(h_out[m], m1, ps3)

            def colsum_sq(h_in, ps_t):
                """ps_t [128,cap] = colsum(h_in^2) replicated on all parts."""
                for m in range(NCH):
                    sq = work.tile([128, cap], bf16, name=f"sq_{m}", tag="sq")
                    nc.vector.tensor_mul(sq, h_in[m], h_in[m])
                    nc.tensor.matmul(
                        ps_t,
                        ones_mat,
                        sq,
                        start=(m == 0),
                        stop=(m == NCH - 1),
                        skip_group_check=True,
                    )

            def down_proj(w2, h_in, ps_o, ps_li=None):
                """oc[:,m,:] (+)= w2.T @ h_in ; ps_o = colsum(o^2) replicated;
                if ps_li given (shared pass): oc += o and ps_li = colsum(oc^2)."""
                for m in range(NCH):
                    po = psum.tile([128, cap], f32, name=f"po_{m}", tag="mm")
                    ks = list(range(1, NCH)) + [0]
                    for j, k in enumerate(ks):
                        nc.tensor.matmul(
                            po, w2(k, m), h_in[k], start=(j == 0), stop=(j == NCH - 1)
                        )
                    if ps_li is not None:
                        sq = work.tile([128, cap], bf16, name=f"sqo_{m}", tag="sq")
                        nc.scalar.square(sq, po)
                        nc.vector.tensor_add(oc[:, m, :], oc[:, m, :], po)
                        sqc = work.tile([128, cap], bf16, name=f"sqc_{m}", tag="sqc")
                        nc.vector.tensor_mul(sqc, oc[:, m, :], oc[:, m, :])
                    else:
                        nc.scalar.copy(oc[:, m, :], po)
                        sq = work.tile([128, cap], bf16, name=f"sqo_{m}", tag="sq")
                        nc.vector.tensor_mul(sq, oc[:, m, :], oc[:, m, :])
                    nc.tensor.matmul(
                        ps_o,
                        ones_mat,
                        sq,
                        start=(m == 0),
                        stop=(m == NCH - 1),
                        skip_group_check=True,
                    )
                    if ps_li is not None:
                        nc.tensor.matmul(
                            ps_li,
                            ones_mat,
                            sqc,
                            start=(m == 0),
                            stop=(m == NCH - 1),
                            skip_group_check=True,
                        )

            # ================= routed expert =================
            w1 = load_w(w1_d, "w1")
            w3 = load_w(w3_d, "w3")
            up_proj(w1, w3, xw_sb, xs_sb, h_r, interleave=True)
            w2 = load_w(w2_d, "w2")
            # r4 load is only needed mid-kernel; keep it off the early queues
            r4_sb = rows.tile([128, cap], f32)
            QS[2].dma_start(out=r4_sb, in_=r4_d[:, :])
            # t' = sqrt(colsum(sp'^2) + (2w)^2) -> bf16 row 0 of h_r[0]
            ps_tr = psum.tile([128, cap], f32, name="ps_tr", tag="mm")
            colsum_sq(h_r, ps_tr)
            trow = rows.tile([1, cap], f32)
            nc.vector.tensor_add(trow, ps_tr[0:1, :], r4_sb[0:1, :])
            nc.scalar.activation(h_r[0][0:1, :], trow, AF.Sqrt)

            ps_or = psum.tile([128, cap], f32, name="ps_or", tag="mm")
            down_proj(w2, h_r, ps_or)

            # ================= shared expert =================
            v1 = load_w(v1_d, "v1")
            v3 = load_w(v3_d, "v3")
            up_proj(v1, v3, xw_sb, xw_sb, h_s)
            v2 = load_w(v2_d, "v2")
            ps_ts = psum.tile([128, cap], f32, name="ps_ts", tag="mm")
            colsum_sq(h_s, ps_ts)
            nc.scalar.activation(h_s[0][0:1, :], ps_ts[0:1, :], AF.Sqrt, bias=1.0)

            ps_os = psum.tile([128, cap], f32, name="ps_os", tag="mm")
            ps_li = psum.tile([128, cap], f32, name="ps_li", tag="mm")
            down_proj(v2, h_s, ps_os, ps_li=ps_li)

            # ================= combine + Lorentz normalize =================
            # (all row quantities replicated across 128 partitions; all
            #  sqrt/rsqrt in the tail via the Abs_reciprocal_sqrt table so
            #  no activation-table reload lands on the critical chain)
            # comb_space = o_shared + 2w*o_routed  (already summed in oc)
            # comb_time  = ot_shared + 2 + 2w*ot_routed
            a_or = rows.tile([128, cap], f32)
            nc.vector.tensor_add(a_or, ps_or, r4_sb)
            r_or = rows.tile([128, cap], f32)
            nc.scalar.activation(r_or, a_or, AF.Abs_reciprocal_sqrt)
            orow = rows.tile([128, cap], f32)
            nc.vector.tensor_mul(orow, a_or, r_or)  # = 2w*ot_routed
            a_os = rows.tile([128, cap], f32)
            nc.vector.tensor_scalar_add(a_os, ps_os, 1.0)
            r_os = rows.tile([128, cap], f32)
            nc.scalar.activation(r_os, a_os, AF.Abs_reciprocal_sqrt)
            osrow = rows.tile([128, cap], f32)
            nc.vector.tensor_mul(osrow, a_os, r_os)  # ot_shared
            ct = rows.tile([128, cap], f32)
            nc.vector.scalar_tensor_tensor(
                ct, orow, 2.0, osrow, op0=ALU.add, op1=ALU.add
            )
            ct2 = rows.tile([128, cap], f32)
            nc.vector.tensor_mul(ct2, ct, ct)
            absli = rows.tile([128, cap], f32)
            nc.vector.tensor_sub(absli, ct2, ps_li)  # = |<comb,comb>_L| > 0
            inv = rows.tile([128, cap], f32)
            nc.scalar.activation(inv, absli, AF.Abs_reciprocal_sqrt)
            otime = rows.tile([1, cap], f32)
            nc.vector.tensor_mul(otime, ct[0:1, :], inv[0:1, :])

            # batched scale by inv (free-dim broadcast), quartered so the
            # output DMAs overlap the remaining scales
            import concourse.bass as bass

            inv_b2 = bass.AP(
                tensor=inv.tensor,
                offset=inv.offset,
                ap=[inv.ap[0], [0, 2], inv.ap[1]],
            )
            for q in range(4):
                sl = slice(2 * q, 2 * q + 2)
                nc.vector.tensor_mul(oc[:, sl, :], oc[:, sl, :], inv_b2)
                if q == 0:
                    nc.vector.tensor_copy(oc[0:1, 0, :], otime)
                QS[q % 3].dma_start(out=out_d[:, sl, :], in_=oc[:, sl, :])

    nc.compile()
    return nc


def _get_nc(cap):
    if cap not in _cache:
        _cache[cap] = _build_nc(cap)
    return _cache[cap]


def _pack_w(mat_t):
    """[1024, 1024] (K, M) -> [128, 8, 1024] partition-major bf16."""
    return np.ascontiguousarray(mat_t.reshape(NCH, 128, D).transpose(1, 0, 2))


def kernel(x, gate_w, gate_b, W1, W3, W2, Ws1, Ws3, Ws2):
    import ml_dtypes

    from concourse.bass_utils import run_bass_kernel_spmd

    bf16 = ml_dtypes.bfloat16
    x = np.asarray(x, dtype=np.float32)
    gate_w = np.asarray(gate_w, dtype=np.float32)
    gate_b = np.asarray(gate_b, dtype=np.float32)
    W1 = np.asarray(W1, dtype=np.float32)
    W3 = np.asarray(W3, dtype=np.float32)
    W2 = np.asarray(W2, dtype=np.float32)
    Ws1 = np.asarray(Ws1, dtype=np.float32)
    Ws3 = np.asarray(Ws3, dtype=np.float32)
    Ws2 = np.asarray(Ws2, dtype=np.float32)

    T = x.shape[0]
    idx, w = _host_gate(x, gate_w, gate_b)
    counts = np.bincount(idx, minlength=E)
    cap = max(64, _round_up(int(counts.max()), 8))
    assert cap <= 512, f"capacity {cap} exceeds single-block limit"

    toks = [np.where(idx == c)[0] for c in range(E)]

    f8 = ml_dtypes.float8_e4m3
    up_dt = f8 if UP_FP8 else bf16

    # host-side weight prep: transposed [K, M] with zero col for the time
    # slot, then packed partition-major
    def prep(Wm, dt=bf16):  # [A, B] -> [B, A+1] (col 0 zero), packed
        out = np.zeros((D, D), dtype=dt)
        out[:, 1:] = Wm.T.astype(dt)
        return _pack_w(out)

    v1t = prep(Ws1, up_dt)
    v3t = prep(Ws3, up_dt)
    v2t = prep(Ws2)

    in_maps = []
    for c in range(E):
        tc_ = toks[c]
        n = len(tc_)
        xt = np.zeros((NCH, 128, cap), dtype=bf16)
        xt.reshape(D, cap)[:, :n] = x[tc_].T.astype(bf16)
        xs = np.zeros((NCH, 128, cap), dtype=bf16)
        tw = (SCALE * w[tc_]).astype(np.float32)
        xs.reshape(D, cap)[:, :n] = (x[tc_] * tw[:, None]).T.astype(bf16)
        r4row = np.zeros((cap,), dtype=np.float32)
        r4row[:n] = tw * tw
        r4 = np.ascontiguousarray(np.broadcast_to(r4row, (128, cap)))
        in_maps.append(
            {
                "xw": np.ascontiguousarray(xt.transpose(1, 0, 2)),
                "xs": np.ascontiguousarray(xs.transpose(1, 0, 2)),
                "w1t": prep(W1[c], up_dt),
                "w3t": prep(W3[c], up_dt),
                "w2t": prep(W2[c]),
                "v1t": v1t,
                "v3t": v3t,
                "v2t": v2t,
                "r4w2": r4,
            }
        )

    nc = _get_nc(cap)
    res = run_bass_kernel_spmd(nc, in_maps, core_ids=list(range(E)))

    out = np.empty((T, D), dtype=np.float32)
    for c in range(E):
        tc_ = toks[c]
        o = res.results[c]["outT"]  # [128, NCH, cap]
        out[tc_] = o.transpose(1, 0, 2).reshape(D, cap)[:, : len(tc_)].T
    return out


if __name__ == "__main__":
    print("smoke build only")
    _build_nc(312)
    print("built ok")


# revision 28
# speedup vs baseline: 1.3349x; 1.1087x over previous
"""LorentzMoE (top-1 routing, E=8 experts) on 8 Trainium2 NeuronCores.

Strategy (expert-parallel, host control plane):
  - Host computes the gate (softmax + bias + top-1) in numpy, exactly
    mirroring the reference numerics.
  - Tokens are dispatched by expert: core c gets every token routed to
    expert c (padded to a uniform capacity so one SPMD NEFF serves all
    8 cores).  Core c also computes the shared expert for those same
    tokens, the LResNet combine and the Lorentz normalization, so each
    token's full output is produced on a single core.
  - The gate weight is folded into the expert FFN: the W3 matmul
    consumes x pre-scaled by (2*w_tok) on the host, making the routed
    space output 2*w*o directly; the routed time component is
    sqrt(colsum(o'^2) + (2w)^2) which only needs a per-token row.
  - Host scatters per-core outputs back to the original token order.

Device layout: feature-on-partition ("transposed") everywhere.
  Weights are packed host-side as [128, 8, 1024] (partition-major) and
  DMAed as four quarter-matrices spread over four HWDGE queues
  (sync/gpsimd/scalar/vector) so packets are 4KB-contiguous per
  partition and aggregate DMA bandwidth is available from t=0.
  Column sums (over partitions) use ones-vector matmuls accumulated
  across the 8 chunks in PSUM; they are deferred out of the up-proj
  phases so PSUM stays shallow, and the |comb|^2 sum is folded into
  the shared-expert down-proj loop to avoid a serial epilogue.
"""

import numpy as np

D = 1024
E = 8
NCH = 8  # 1024 / 128 partition chunks
SCALE = 2.0
UP_FP8 = True  # fp8e4m3 weights for the four up-projection matrices
DOUBLE_ROW = True  # fp8 DoubleRow: pair k-chunks, halves up-matmul count

_cache: dict = {}


def _round_up(v, m):
    return ((v + m - 1) // m) * m


def _host_gate(x, gate_w, gate_b):
    """Replicates the reference gate in f32 numpy (verified bit-identical
    top-1 selection vs the jax reference on the benchmark inputs)."""
    logits = (x[:, 1:] @ gate_w.T).astype(np.float32)
    m = logits.max(-1, keepdims=True)
    e = np.exp(logits - m, dtype=np.float32)
    scores = e / e.sum(-1, keepdims=True, dtype=np.float32)
    biased = scores + gate_b
    idx = np.argmax(biased, axis=-1)
    w = scores[np.arange(x.shape[0]), idx]
    return idx.astype(np.int64), w.astype(np.float32)


def _build_nc(cap):
    import concourse.mybir as mybir
    import concourse.tile as tile
    from concourse import bacc

    f32 = mybir.dt.float32
    bf16 = mybir.dt.bfloat16
    AF = mybir.ActivationFunctionType
    ALU = mybir.AluOpType

    nc = bacc.Bacc("TRN2", target_bir_lowering=False, debug=False)

    f8 = mybir.dt.float8e4
    up_dt = f8 if UP_FP8 else bf16

    # ---- DRAM I/O (weights packed [128, NCH, 1024]; x [128, NCH, cap]) ----
    if DOUBLE_ROW:
        xw_d = nc.dram_tensor("xw", [128, 4, 2, cap], f8, kind="ExternalInput")
        xs_d = nc.dram_tensor("xs", [128, 4, 2, cap], f8, kind="ExternalInput")
    else:
        xw_d = nc.dram_tensor("xw", [128, NCH, cap], bf16, kind="ExternalInput")
        xs_d = nc.dram_tensor("xs", [128, NCH, cap], bf16, kind="ExternalInput")
    w1_d = nc.dram_tensor("w1t", [128, NCH, D], up_dt, kind="ExternalInput")
    w3_d = nc.dram_tensor("w3t", [128, NCH, D], up_dt, kind="ExternalInput")
    w2_d = nc.dram_tensor("w2t", [128, NCH, D], bf16, kind="ExternalInput")
    v1_d = nc.dram_tensor("v1t", [128, NCH, D], up_dt, kind="ExternalInput")
    v3_d = nc.dram_tensor("v3t", [128, NCH, D], up_dt, kind="ExternalInput")
    v2_d = nc.dram_tensor("v2t", [128, NCH, D], bf16, kind="ExternalInput")
    r4_d = nc.dram_tensor("r4w2", [128, cap], f32, kind="ExternalInput")
    out_d = nc.dram_tensor("outT", [128, NCH, cap], f32, kind="ExternalOutput")

    with tile.TileContext(nc) as tc:
        with (
            tc.tile_pool(name="consts", bufs=1) as consts,
            tc.tile_pool(name="xpool", bufs=1) as xpool,
            tc.tile_pool(name="wpool", bufs=4) as wpool,
            tc.tile_pool(name="hpool", bufs=1) as hpool,
            tc.tile_pool(name="work", bufs=3) as work,
            tc.tile_pool(name="rows", bufs=1) as rows,
            tc.tile_pool(name="psum", bufs=8, space="PSUM") as psum,
        ):
            QS = [nc.sync, nc.gpsimd, nc.scalar]

            ones_mat = consts.tile([128, 128], bf16)
            nc.vector.memset(ones_mat, 1.0)

            # warm up the PE HAM clock-gate with dummy matmuls while the
            # first weight DMAs are in flight (PE would be idle anyway)
            warm_ps = psum.tile([128, 128], f32, name="warm_ps", tag="mm")
            for _ in range(56):
                nc.tensor.matmul(
                    warm_ps, ones_mat, ones_mat, skip_group_check=True
                )

            # x first on each queue so the first matmuls have operands
            if DOUBLE_ROW:
                xw_a = xpool.tile([128, 2, 2, cap], f8)
                QS[0].dma_start(out=xw_a, in_=xw_d[:, 0:2, :, :])
                xw_b = xpool.tile([128, 2, 2, cap], f8)
                QS[1].dma_start(out=xw_b, in_=xw_d[:, 2:4, :, :])
                xs_a = xpool.tile([128, 2, 2, cap], f8)
                QS[2].dma_start(out=xs_a, in_=xs_d[:, 0:2, :, :])
                xs_b = xpool.tile([128, 2, 2, cap], f8)
                QS[2].dma_start(out=xs_b, in_=xs_d[:, 2:4, :, :])
                xw_sb = [(xw_a if c < 2 else xw_b)[:, c % 2, :, :] for c in range(4)]
                xs_sb = [(xs_a if c < 2 else xs_b)[:, c % 2, :, :] for c in range(4)]
            else:
                xw_a = xpool.tile([128, 4, cap], bf16)
                QS[0].dma_start(out=xw_a, in_=xw_d[:, 0:4, :])
                xw_b = xpool.tile([128, 4, cap], bf16)
                QS[1].dma_start(out=xw_b, in_=xw_d[:, 4:8, :])
                xs_a = xpool.tile([128, 4, cap], bf16)
                QS[2].dma_start(out=xs_a, in_=xs_d[:, 0:4, :])
                xs_b = xpool.tile([128, 4, cap], bf16)
                QS[2].dma_start(out=xs_b, in_=xs_d[:, 4:8, :])
                xw_sb = [(xw_a if k < 4 else xw_b)[:, k % 4, :] for k in range(NCH)]
                xs_sb = [(xs_a if k < 4 else xs_b)[:, k % 4, :] for k in range(NCH)]



            _w_count = [0]

            def load_w(dram, nm):
                """Four quarter-matrix tiles spread over the HWDGE queues."""
                qt = []
                off = _w_count[0]
                _w_count[0] += 1
                for q in range(4):
                    t = wpool.tile(
                        [128, 2, D], dram.dtype, name=f"{nm}q{q}", tag=f"wq{q}"
                    )
                    QS[(q + off) % 3].dma_start(
                        out=t, in_=dram[:, 2 * q : 2 * q + 2, :]
                    )
                    qt.append(t)

                def sl(k, m):
                    return qt[k // 2][:, k % 2, 128 * m : 128 * (m + 1)]

                def sl_pair(c, m):
                    return qt[c][:, :, 128 * m : 128 * (m + 1)]

                sl.pair = sl_pair
                return sl

            # persistent activations
            h_r = [
                hpool.tile([128, cap], bf16, name=f"hr{k}", tag=f"hr{k}")
                for k in range(NCH)
            ]
            h_s = [
                hpool.tile([128, cap], bf16, name=f"hs{k}", tag=f"hs{k}")
                for k in range(NCH)
            ]
            oc = hpool.tile([128, NCH, cap], f32)

            DR = mybir.MatmulPerfMode.DoubleRow

            def up_proj(wa, wb, xa, xb, h_out, interleave=False):
                """h_out[m] = bf16( silu(wa.T@xa) * (wb.T@xb) ) per chunk m.
                DoubleRow: contraction in 4 fp8 k-chunk pairs.
                interleave=True orders the loops by DMA quarter arrival
                so the first phase can start before all weights landed."""
                for m in range(NCH):
                    ps1 = psum.tile([128, cap], f32, name=f"ps1_{m}", tag="mm")
                    ps3 = psum.tile([128, cap], f32, name=f"ps3_{m}", tag="mm")
                    if DOUBLE_ROW:
                        order = (
                            [(ps1, wa, xa, 0), (ps1, wa, xa, 1),
                             (ps3, wb, xb, 0), (ps3, wb, xb, 1),
                             (ps1, wa, xa, 2), (ps1, wa, xa, 3),
                             (ps3, wb, xb, 2), (ps3, wb, xb, 3)]
                            if interleave
                            else [(ps1, wa, xa, c) for c in range(4)]
                            + [(ps3, wb, xb, c) for c in range(4)]
                        )
                        seen = {}
                        for ps, wf, xf, c in order:
                            first = id(ps) not in seen
                            seen[id(ps)] = seen.get(id(ps), 0) + 1
                            nc.tensor.matmul(
                                ps, wf.pair(c, m), xf[c],
                                start=first, stop=(seen[id(ps)] == 4),
                                perf_mode=DR, skip_group_check=True,
                            )
                    elif interleave:
                        for k in range(4):
                            nc.tensor.matmul(
                                ps1, wa(k, m), xa[k], start=(k == 0), stop=False,
                                skip_group_check=True,
                            )
                        for k in range(4):
                            nc.tensor.matmul(
                                ps3, wb(k, m), xb[k], start=(k == 0), stop=False,
                                skip_group_check=True,
                            )
                        for k in range(4, NCH):
                            nc.tensor.matmul(
                                ps1, wa(k, m), xa[k], start=False,
                                stop=(k == NCH - 1), skip_group_check=True,
                            )
                        for k in range(4, NCH):
                            nc.tensor.matmul(
                                ps3, wb(k, m), xb[k], start=False,
                                stop=(k == NCH - 1), skip_group_check=True,
                            )
                    else:
                        for k in range(NCH):
                            nc.tensor.matmul(
                                ps1, wa(k, m), xa[k], start=(k == 0),
                                stop=(k == NCH - 1),
                            )
                        for k in range(NCH):
                            nc.tensor.matmul(
                                ps3, wb(k, m), xb[k], start=(k == 0),
                                stop=(k == NCH - 1),
                            )
                    m1 = work.tile([128, cap], f32, name=f"m1_{m}", tag="m1")
                    nc.scalar.activation(m1, ps1, AF.Silu)
                    nc.vector.tensor_mul(h_out[m], m1, ps3)

            def colsum_sq(h_in, ps_t):
                """ps_t [128,cap] = colsum(h_in^2) replicated on all parts."""
                for m in range(NCH):
                    sq = work.tile([128, cap], bf16, name=f"sq_{m}", tag="sq")
                    nc.vector.tensor_mul(sq, h_in[m], h_in[m])
                    nc.tensor.matmul(
                        ps_t,
                        ones_mat,
                        sq,
                        start=(m == 0),
                        stop=(m == NCH - 1),
                        skip_group_check=True,
                    )

            def down_proj(w2, h_in, ps_o, ps_li=None):
                """oc[:,m,:] (+)= w2.T @ h_in ; ps_o = colsum(o^2) replicated;
                if ps_li given (shared pass): oc += o and ps_li = colsum(oc^2)."""
                for m in range(NCH):
                    po = psum.tile([128, cap], f32, name=f"po_{m}", tag="mm")
                    ks = list(range(1, NCH)) + [0]
                    for j, k in enumerate(ks):
                        nc.tensor.matmul(
                            po, w2(k, m), h_in[k], start=(j == 0), stop=(j == NCH - 1)
                        )
                    if ps_li is not None:
                        sq = work.tile([128, cap], bf16, name=f"sqo_{m}", tag="sq")
                        nc.scalar.square(sq, po)
                        nc.vector.tensor_add(oc[:, m, :], oc[:, m, :], po)
                        sqc = work.tile([128, cap], bf16, name=f"sqc_{m}", tag="sqc")
                        nc.vector.tensor_mul(sqc, oc[:, m, :], oc[:, m, :])
                    else:
                        nc.scalar.copy(oc[:, m, :], po)
                        sq = work.tile([128, cap], bf16, name=f"sqo_{m}", tag="sq")
                        nc.vector.tensor_mul(sq, oc[:, m, :], oc[:, m, :])
                    nc.tensor.matmul(
                        ps_o,
                        ones_mat,
                        sq,
                        start=(m == 0),
                        stop=(m == NCH - 1),
                        skip_group_check=True,
                    )
                    if ps_li is not None:
                        nc.tensor.matmul(
                            ps_li,
                            ones_mat,
                            sqc,
                            start=(m == 0),
                            stop=(m == NCH - 1),
                            skip_group_check=True,
                        )

            # ================= routed expert =================
            w1 = load_w(w1_d, "w1")
            w3 = load_w(w3_d, "w3")
            up_proj(w1, w3, xw_sb, xs_sb, h_r, interleave=True)
            w2 = load_w(w2_d, "w2")
            # r4 load is only needed mid-kernel; keep it off the early queues
            r4_sb = rows.tile([128, cap], f32)
            QS[2].dma_start(out=r4_sb, in_=r4_d[:, :])
            # t' = sqrt(colsum(sp'^2) + (2w)^2) -> bf16 row 0 of h_r[0]
            ps_tr = psum.tile([128, cap], f32, name="ps_tr", tag="mm")
            colsum_sq(h_r, ps_tr)
            trow = rows.tile([1, cap], f32)
            nc.vector.tensor_add(trow, ps_tr[0:1, :], r4_sb[0:1, :])
            nc.scalar.activation(h_r[0][0:1, :], trow, AF.Sqrt)

            ps_or = psum.tile([128, cap], f32, name="ps_or", tag="mm")
            down_proj(w2, h_r, ps_or)

            # ================= shared expert =================
            v1 = load_w(v1_d, "v1")
            v3 = load_w(v3_d, "v3")
            up_proj(v1, v3, xw_sb, xw_sb, h_s)
            v2 = load_w(v2_d, "v2")
            ps_ts = psum.tile([128, cap], f32, name="ps_ts", tag="mm")
            colsum_sq(h_s, ps_ts)
            nc.scalar.activation(h_s[0][0:1, :], ps_ts[0:1, :], AF.Sqrt, bias=1.0)

            ps_os = psum.tile([128, cap], f32, name="ps_os", tag="mm")
            ps_li = psum.tile([128, cap], f32, name="ps_li", tag="mm")
            down_proj(v2, h_s, ps_os, ps_li=ps_li)

            # ================= combine + Lorentz normalize =================
            # (all row quantities replicated across 128 partitions; all
            #  sqrt/rsqrt in the tail via the Abs_reciprocal_sqrt table so
            #  no activation-table reload lands on the critical chain)
            # comb_space = o_shared + 2w*o_routed  (already summed in oc)
            # comb_time  = ot_shared + 2 + 2w*ot_routed
            a_or = rows.tile([128, cap], f32)
            nc.vector.tensor_add(a_or, ps_or, r4_sb)
            r_or = rows.tile([128, cap], f32)
            nc.scalar.activation(r_or, a_or, AF.Abs_reciprocal_sqrt)
            orow = rows.tile([128, cap], f32)
            nc.vector.tensor_mul(orow, a_or, r_or)  # = 2w*ot_routed
            a_os = rows.tile([128, cap], f32)
            nc.vector.tensor_scalar_add(a_os, ps_os, 1.0)
            r_os = rows.tile([128, cap], f32)
            nc.scalar.activation(r_os, a_os, AF.Abs_reciprocal_sqrt)
            osrow = rows.tile([128, cap], f32)
            nc.vector.tensor_mul(osrow, a_os, r_os)  # ot_shared
            ct = rows.tile([128, cap], f32)
            nc.vector.scalar_tensor_tensor(
                ct, orow, 2.0, osrow, op0=ALU.add, op1=ALU.add
            )
            ct2 = rows.tile([128, cap], f32)
            nc.vector.tensor_mul(ct2, ct, ct)
            absli = rows.tile([128, cap], f32)
            nc.vector.tensor_sub(absli, ct2, ps_li)  # = |<comb,comb>_L| > 0
            inv = rows.tile([128, cap], f32)
            nc.scalar.activation(inv, absli, AF.Abs_reciprocal_sqrt)
            otime = rows.tile([1, cap], f32)
            nc.vector.tensor_mul(otime, ct[0:1, :], inv[0:1, :])

            # batched scale by inv (free-dim broadcast), quartered so the
            # output DMAs overlap the remaining scales
            import concourse.bass as bass

            inv_b2 = bass.AP(
                tensor=inv.tensor,
                offset=inv.offset,
                ap=[inv.ap[0], [0, 2], inv.ap[1]],
            )
            for q in range(4):
                sl = slice(2 * q, 2 * q + 2)
                nc.vector.tensor_mul(oc[:, sl, :], oc[:, sl, :], inv_b2)
                if q == 0:
                    nc.vector.tensor_copy(oc[0:1, 0, :], otime)
                QS[q % 3].dma_start(out=out_d[:, sl, :], in_=oc[:, sl, :])

    nc.compile()
    return nc


def _get_nc(cap):
    if cap not in _cache:
        _cache[cap] = _build_nc(cap)
    return _cache[cap]


def _pack_w(mat_t):
    """[1024, 1024] (K, M) -> [128, 8, 1024] partition-major bf16."""
    return np.ascontiguousarray(mat_t.reshape(NCH, 128, D).transpose(1, 0, 2))


def kernel(x, gate_w, gate_b, W1, W3, W2, Ws1, Ws3, Ws2):
    import ml_dtypes

    from concourse.bass_utils import run_bass_kernel_spmd

    bf16 = ml_dtypes.bfloat16
    x = np.asarray(x, dtype=np.float32)
    gate_w = np.asarray(gate_w, dtype=np.float32)
    gate_b = np.asarray(gate_b, dtype=np.float32)
    W1 = np.asarray(W1, dtype=np.float32)
    W3 = np.asarray(W3, dtype=np.float32)
    W2 = np.asarray(W2, dtype=np.float32)
    Ws1 = np.asarray(Ws1, dtype=np.float32)
    Ws3 = np.asarray(Ws3, dtype=np.float32)
    Ws2 = np.asarray(Ws2, dtype=np.float32)

    T = x.shape[0]
    idx, w = _host_gate(x, gate_w, gate_b)
    counts = np.bincount(idx, minlength=E)
    cap = max(64, _round_up(int(counts.max()), 8))
    assert cap <= 512, f"capacity {cap} exceeds single-block limit"

    toks = [np.where(idx == c)[0] for c in range(E)]

    f8 = ml_dtypes.float8_e4m3
    up_dt = f8 if UP_FP8 else bf16
    x_dt = f8 if DOUBLE_ROW else bf16

    # host-side weight prep: transposed [K, M] with zero col for the time
    # slot, then packed partition-major
    def prep(Wm, dt=bf16):  # [A, B] -> [B, A+1] (col 0 zero), packed
        out = np.zeros((D, D), dtype=dt)
        out[:, 1:] = Wm.T.astype(dt)
        return _pack_w(out)

    v1t = prep(Ws1, up_dt)
    v3t = prep(Ws3, up_dt)
    v2t = prep(Ws2)

    in_maps = []
    for c in range(E):
        tc_ = toks[c]
        n = len(tc_)
        xt = np.zeros((NCH, 128, cap), dtype=x_dt)
        xt.reshape(D, cap)[:, :n] = x[tc_].T.astype(x_dt)
        xs = np.zeros((NCH, 128, cap), dtype=x_dt)
        tw = (SCALE * w[tc_]).astype(np.float32)
        xs.reshape(D, cap)[:, :n] = (x[tc_] * tw[:, None]).T.astype(x_dt)
        if DOUBLE_ROW:
            xt = xt.reshape(4, 2, 128, cap).transpose(2, 0, 1, 3)
            xs = xs.reshape(4, 2, 128, cap).transpose(2, 0, 1, 3)
            in_maps_x = None  # marker unused
        r4row = np.zeros((cap,), dtype=np.float32)
        r4row[:n] = tw * tw
        r4 = np.ascontiguousarray(np.broadcast_to(r4row, (128, cap)))
        in_maps.append(
            {
                "xw": np.ascontiguousarray(
                    xt if DOUBLE_ROW else xt.transpose(1, 0, 2)
                ),
                "xs": np.ascontiguousarray(
                    xs if DOUBLE_ROW else xs.transpose(1, 0, 2)
                ),
                "w1t": prep(W1[c], up_dt),
                "w3t": prep(W3[c], up_dt),
                "w2t": prep(W2[c]),
                "v1t": v1t,
                "v3t": v3t,
                "v2t": v2t,
                "r4w2": r4,
            }
        )

    nc = _get_nc(cap)
    res = run_bass_kernel_spmd(nc, in_maps, core_ids=list(range(E)))

    out = np.empty((T, D), dtype=np.float32)
    for c in range(E):
        tc_ = toks[c]
        o = res.results[c]["outT"]  # [128, NCH, cap]
        out[tc_] = o.transpose(1, 0, 2).reshape(D, cap)[:, : len(tc_)].T
    return out


if __name__ == "__main__":
    print("smoke build only")
    _build_nc(312)
    print("built ok")
